# revision 3
# baseline (speedup 1.0000x reference)
"""ImprovedGRUCell Trainium2 kernel (8-core data-parallel over batch).

v2 layout strategy — fully transposed (feature-major) compute:
  - batch sharded 8 ways (8192 rows/core); 256x256 weights replicated.
  - Host pre-transposes x/h to [256, B] bf16 per core, so activations load
    feature-major directly: SBUF tiles [128 part = feature, free = batch].
    No on-device transposes at all.
  - All GEMMs run in transposed orientation S^T[h, b] = W @ x^T with the
    weight tile as the stationary operand (lhsT = W.T block [128k, 128h])
    and the activation as the moving operand (N=512 free).  24 matmuls of
    [128,128]x[128,512] per 512-column super-tile.
  - softmax over hidden: va scale + exp are per-partition (ACT); the
    denominator is a gpsimd partition_all_reduce (no tensor-engine ops),
    already replicated across partitions for the normalize multiply.
    Normalization is applied to h before the att = E * (h * 1/denom)
    product, which then feeds the U_h matmul directly.
  - sigmoid avoided: z = 0.5*tanh(s/2)+0.5 folded into the blend:
    h_t = h + 0.5*(tz+1)*(htl-h), computed as two scalar_tensor_tensor ops.
    ACT uses only {Tanh, Exp} (one table set, no reloads).
  - Software pipelining: the candidate branch + blend of super-tile i are
    emitted two iterations later, so the PE never head-of-line blocks on
    the softmax chain.
  - Output written bf16 transposed [256, B]; host transposes/casts to f32.
"""

import sys

sys.path.insert(0, "/opt/trn_rl_repo")

import ml_dtypes
import numpy as np

import concourse.bass as bass
import concourse.mybir as mybir
from concourse import bacc, tile
from concourse.bass_utils import run_bass_kernel_spmd

B_TOTAL = 65536
N_CORES = 8
B_CORE = B_TOTAL // N_CORES  # 8192
D = 256
ST = 512  # batch columns per super-tile
N_ST = B_CORE // ST  # 16
LAG = 2  # software-pipeline depth for candidate branch + blend

F32 = mybir.dt.float32
BF16 = mybir.dt.bfloat16
AF = mybir.ActivationFunctionType
ALU = mybir.AluOpType
RED = bass.bass_isa.ReduceOp

_CACHE = {}

WNAMES = ("wzt", "uzt", "wat", "uat", "wht", "uht")


def build_nc(use_bias=False):
    nc = bacc.Bacc(
        "TRN2",
        target_bir_lowering=False,
        debug=False,
        enable_asserts=False,
        num_devices=N_CORES,
    )

    x_d = nc.dram_tensor("xT", [D, B_CORE], BF16, kind="ExternalInput")
    h_d = nc.dram_tensor("hT", [D, B_CORE], BF16, kind="ExternalInput")
    w_d = {
        n: nc.dram_tensor(n, [D, D], BF16, kind="ExternalInput") for n in WNAMES
    }
    va_d = nc.dram_tensor("va", [D], F32, kind="ExternalInput")
    bz_d = nc.dram_tensor("bzh", [128, 2], F32, kind="ExternalInput")
    bh_d = nc.dram_tensor("bh", [128, 2], F32, kind="ExternalInput")
    out_d = nc.dram_tensor("out", [D, B_CORE], BF16, kind="ExternalOutput")

    with tile.TileContext(nc) as tc:
        with (
            tc.tile_pool(name="wpool", bufs=1) as wp,
            tc.tile_pool(name="io", bufs=4) as io,
            tc.tile_pool(name="wk", bufs=2) as wk,
            tc.tile_pool(name="psz", bufs=1, space="PSUM") as psZ,
            tc.tile_pool(name="psa", bufs=2, space="PSUM") as psA,
            tc.tile_pool(name="psc", bufs=1, space="PSUM") as psC,
        ):
            # ---- persistent weights -------------------------------------
            # w_sb[n][p, kb*D + h] = W.T[kb*128 + p, h]; lhsT tile for
            # (kb, hb) = w_sb[:, kb*D + hb*128 :][:128].
            w_sb = {}
            for n in WNAMES:
                t = wp.tile([128, 2 * D], BF16, tag=n, name=n)
                nc.sync.dma_start(
                    out=t.rearrange("p (kb h) -> p kb h", kb=2),
                    in_=w_d[n].ap().rearrange("(kb p) h -> p kb h", p=128),
                )
                w_sb[n] = t

            def wsl(n, kb, hb):
                e = kb * D + hb * 128
                return w_sb[n][:, e : e + 128]

            va_sb = wp.tile([128, 2], F32, tag="va")
            nc.sync.dma_start(
                out=va_sb[:], in_=va_d.ap().rearrange("(t p) -> p t", p=128)
            )
            if use_bias:
                bz_sb = wp.tile([128, 2], F32, tag="bz")
                nc.sync.dma_start(out=bz_sb[:], in_=bz_d.ap())
                bh_sb = wp.tile([128, 2], F32, tag="bh")
                nc.sync.dma_start(out=bh_sb[:], in_=bh_d.ap())

            # cross-iteration tile refs for the software pipeline
            xts, hts, tzs, atts = {}, {}, {}, {}

            for it in range(N_ST + LAG):
                # ==== stage A: loads + z/attention branches for tile `it`
                if it < N_ST:
                    b0 = it * ST
                    xt = io.tile([128, 2 * ST], BF16, tag="xt", bufs=4)
                    nc.sync.dma_start(
                        out=xt.rearrange("p (kb b) -> p kb b", kb=2),
                        in_=x_d.ap()[:, b0 : b0 + ST].rearrange(
                            "(kb p) b -> p kb b", p=128
                        ),
                    )
                    ht = io.tile([128, 2 * ST], BF16, tag="ht", bufs=4)
                    nc.sync.dma_start(
                        out=ht.rearrange("p (kb b) -> p kb b", kb=2),
                        in_=h_d.ap()[:, b0 : b0 + ST].rearrange(
                            "(kb p) b -> p kb b", p=128
                        ),
                    )
                    xts[it], hts[it] = xt, ht

                # ==== stage C (lagged): candidate branch + blend for `jt`
                jt = it - LAG
                if jt >= 0:
                    xtj, htj = xts.pop(jt), hts[jt]
                    tzj, attj = tzs.pop(jt), atts.pop(jt)
                    pc = psC.tile([128, 2 * ST], F32, tag="pc")
                    for hb in range(2):
                        o = pc[:, hb * ST : (hb + 1) * ST]
                        for kb in range(2):
                            nc.tensor.matmul(
                                o,
                                wsl("wht", kb, hb),
                                xtj[:, kb * ST : (kb + 1) * ST],
                                start=(kb == 0),
                                stop=False,
                            )
                        for kb in range(2):
                            nc.tensor.matmul(
                                o,
                                wsl("uht", kb, hb),
                                attj[:, kb * ST : (kb + 1) * ST],
                                start=False,
                                stop=(kb == 1),
                            )
                    htl = wk.tile([128, 2 * ST], BF16, tag="htl")
                    if use_bias:
                        for hb in range(2):
                            sl = slice(hb * ST, (hb + 1) * ST)
                            nc.scalar.activation(
                                htl[:, sl], pc[:, sl], AF.Tanh,
                                bias=bh_sb[:, hb : hb + 1],
                            )
                    else:
                        nc.scalar.activation(htl[:], pc[:], AF.Tanh)
                    # blend: h_t = h + 0.5*(tz+1)*(htl-h)
                    dd = wk.tile([128, 2 * ST], BF16, tag="dd")
                    nc.gpsimd.tensor_sub(dd[:], htl[:], htj[:])
                    pp = wk.tile([128, 2 * ST], BF16, tag="pp")
                    nc.vector.scalar_tensor_tensor(
                        pp[:], tzj[:], 1.0, dd[:], op0=ALU.add, op1=ALU.mult
                    )
                    ot = io.tile([128, 2 * ST], BF16, tag="ot", bufs=3)
                    nc.vector.scalar_tensor_tensor(
                        ot[:], pp[:], 0.5, htj[:], op0=ALU.mult, op1=ALU.add
                    )
                    del hts[jt]
                    nc.sync.dma_start(
                        out=out_d.ap()[:, jt * ST : (jt + 1) * ST].rearrange(
                            "(hb p) b -> p hb b", p=128
                        ),
                        in_=ot.rearrange("p (hb b) -> p hb b", hb=2),
                    )

                if it >= N_ST:
                    continue

                # ==== stage B: attention + z branches for tile `it`
                pa = psA.tile([128, 2 * ST], F32, tag="pa")
                for hb in range(2):
                    o = pa[:, hb * ST : (hb + 1) * ST]
                    for kb in range(2):
                        nc.tensor.matmul(
                            o,
                            wsl("wat", kb, hb),
                            xt[:, kb * ST : (kb + 1) * ST],
                            start=(kb == 0),
                            stop=False,
                        )
                    for kb in range(2):
                        nc.tensor.matmul(
                            o,
                            wsl("uat", kb, hb),
                            ht[:, kb * ST : (kb + 1) * ST],
                            start=False,
                            stop=(kb == 1),
                        )
                A = wk.tile([128, 2 * ST], BF16, tag="A")
                nc.scalar.activation(A[:], pa[:], AF.Tanh)
                E = wk.tile([128, 2 * ST], BF16, tag="E")
                for hb in range(2):
                    sl = slice(hb * ST, (hb + 1) * ST)
                    nc.scalar.activation(
                        E[:, sl], A[:, sl], AF.Exp, scale=va_sb[:, hb : hb + 1]
                    )

                # z branch matmuls (independent; keeps PE busy during softmax)
                pz = psZ.tile([128, 2 * ST], F32, tag="pz")
                for hb in range(2):
                    o = pz[:, hb * ST : (hb + 1) * ST]
                    for kb in range(2):
                        nc.tensor.matmul(
                            o,
                            wsl("wzt", kb, hb),
                            xt[:, kb * ST : (kb + 1) * ST],
                            start=(kb == 0),
                            stop=False,
                        )
                    for kb in range(2):
                        nc.tensor.matmul(
                            o,
                            wsl("uzt", kb, hb),
                            ht[:, kb * ST : (kb + 1) * ST],
                            start=False,
                            stop=(kb == 1),
                        )
                tz = wk.tile([128, 2 * ST], BF16, tag="tz", bufs=3)
                if use_bias:
                    for hb in range(2):
                        sl = slice(hb * ST, (hb + 1) * ST)
                        nc.scalar.activation(
                            tz[:, sl], pz[:, sl], AF.Tanh,
                            bias=bz_sb[:, hb : hb + 1], scale=0.5,
                        )
                else:
                    nc.scalar.activation(tz[:], pz[:], AF.Tanh, scale=0.5)
                tzs[it] = tz

                # softmax denominator + normalized attended_h (transposed)
                es = wk.tile([128, ST], F32, tag="es")
                nc.vector.tensor_add(es[:], E[:, 0:ST], E[:, ST : 2 * ST])
                ds = wk.tile([128, ST], F32, tag="ds")
                nc.gpsimd.partition_all_reduce(ds[:], es[:], 128, RED.add)
                rb = wk.tile([128, ST], F32, tag="rb")
                nc.vector.reciprocal(rb[:], ds[:])
                hr = wk.tile([128, 2 * ST], BF16, tag="hr")
                for kb in range(2):
                    sl = slice(kb * ST, (kb + 1) * ST)
                    nc.vector.tensor_mul(hr[:, sl], ht[:, sl], rb[:])
                att = wk.tile([128, 2 * ST], BF16, tag="att", bufs=3)
                nc.vector.tensor_mul(att[:], E[:], hr[:])
                atts[it] = att

    nc.compile()
    return nc


LAST_RESULTS = None


def kernel(x, h_prev, W_z, U_z, b_z, W_a, U_a, v_a, W_h, U_h, b_h):
    global LAST_RESULTS
    use_bias = bool(np.any(np.asarray(b_z)) or np.any(np.asarray(b_h)))
    key = ("nc", use_bias)
    if key not in _CACHE:
        _CACHE[key] = build_nc(use_bias)
    nc = _CACHE[key]

    bf = ml_dtypes.bfloat16
    xbf = np.asarray(x, dtype=np.float32).astype(bf)
    hbf = np.asarray(h_prev, dtype=np.float32).astype(bf)
    wmats = {
        "wzt": W_z,
        "uzt": U_z,
        "wat": W_a,
        "uat": U_a,
        "wht": W_h,
        "uht": U_h,
    }
    common = {
        n: np.ascontiguousarray(np.asarray(m, dtype=np.float32).T.astype(bf))
        for n, m in wmats.items()
    }
    common["va"] = np.ascontiguousarray(np.asarray(v_a, dtype=np.float32))
    # biases laid out [128 partition, 2 h-block]; z bias pre-scaled by 0.5
    common["bzh"] = np.ascontiguousarray(
        (0.5 * np.asarray(b_z, dtype=np.float32)).reshape(2, 128).T
    )
    common["bh"] = np.ascontiguousarray(
        np.asarray(b_h, dtype=np.float32).reshape(2, 128).T
    )

    in_maps = []
    for c in range(N_CORES):
        m = dict(common)
        m["xT"] = np.ascontiguousarray(xbf[c * B_CORE : (c + 1) * B_CORE].T)
        m["hT"] = np.ascontiguousarray(hbf[c * B_CORE : (c + 1) * B_CORE].T)
        in_maps.append(m)

    LAST_RESULTS = run_bass_kernel_spmd(nc, in_maps, core_ids=list(range(N_CORES)))
    outs = LAST_RESULTS.results
    res = np.empty((B_TOTAL, D), np.float32)
    for c in range(N_CORES):
        res[c * B_CORE : (c + 1) * B_CORE] = outs[c]["out"].T
    return res


# revision 6
# speedup vs baseline: 2.4974x; 2.4974x over previous
"""ImprovedGRUCell Trainium2 kernel (8-core data-parallel over batch).

v3 layout strategy — fully transposed (feature-major) compute:
  - batch sharded 8 ways (8192 rows/core); 256x256 weights replicated.
  - Host pre-transposes x/h to [256, B] bf16 per core, so activations load
    feature-major directly: SBUF tiles [128 part = feature, free = batch].
    No on-device transposes.
  - All GEMMs in transposed orientation S^T[h, b] = W @ x^T: weight tile
    stationary (lhsT = W.T block [128k, 128h]), activation moving (N=512).
    24 matmuls of [128,128]x[128,512] per 512-column super-tile.
  - softmax over hidden (partition dim): va scale + exp are per-partition
    ACT ops.  Denominator: ones_col.T @ E -> [1,512] psum row; gpsimd
    copies it to SBUF; rank-1 matmul (ones_row x row) re-broadcasts to
    [128,512] psum; reciprocal_approx_fast gives rb.  Normalization is
    folded into h before att = E * (h * rb), which feeds U_h directly.
  - sigmoid avoided: z = 0.5*tanh(s/2)+0.5; blend h_t = h + u*(htl-h)
    with u = 0.5*tz+0.5 via tensor_scalar + two all-bf16 tensor_tensor
    ops (DVE 2x packed mode needs every operand 2-byte).
  - 3-deep software pipeline: iter `it` emits loads(it)+z/a-branch(it),
    denom-bcast/normalize(it-1), candidate+tanh+sub(it-2), blend+store
    (it-3), so every engine FIFO sees only ready work and the PE never
    idles long enough to re-throttle (HAM).
  - Output written bf16 transposed [256, B]; host transposes/casts to f32.
"""

import sys

sys.path.insert(0, "/opt/trn_rl_repo")

import ml_dtypes
import numpy as np

import concourse.bass as bass
import concourse.mybir as mybir
from concourse import bacc, tile
from concourse.bass_utils import run_bass_kernel_spmd

B_TOTAL = 65536
N_CORES = 8
B_CORE = B_TOTAL // N_CORES  # 8192
D = 256
ST = 512  # batch columns per super-tile
N_ST = B_CORE // ST  # 16

F32 = mybir.dt.float32
BF16 = mybir.dt.bfloat16
AF = mybir.ActivationFunctionType
ALU = mybir.AluOpType

_CACHE = {}

WNAMES = ("wzt", "uzt", "wat", "uat", "wht", "uht")


def build_nc(use_bias=False):
    nc = bacc.Bacc(
        "TRN2",
        target_bir_lowering=False,
        debug=False,
        enable_asserts=False,
        num_devices=N_CORES,
    )

    x_d = nc.dram_tensor("xT", [D, B_CORE], BF16, kind="ExternalInput")
    h_d = nc.dram_tensor("hT", [D, B_CORE], BF16, kind="ExternalInput")
    w_d = {
        n: nc.dram_tensor(n, [D, D], BF16, kind="ExternalInput") for n in WNAMES
    }
    va_d = nc.dram_tensor("va", [D], F32, kind="ExternalInput")
    bz_d = nc.dram_tensor("bzh", [128, 2], F32, kind="ExternalInput")
    bh_d = nc.dram_tensor("bh", [128, 2], F32, kind="ExternalInput")
    out_d = nc.dram_tensor("out", [D, B_CORE], BF16, kind="ExternalOutput")

    with tile.TileContext(nc) as tc:
        with (
            tc.tile_pool(name="wpool", bufs=1) as wp,
            tc.tile_pool(name="io", bufs=4) as io,
            tc.tile_pool(name="wk", bufs=2) as wk,
            tc.tile_pool(name="psz", bufs=1, space="PSUM") as psZ,
            tc.tile_pool(name="psa", bufs=1, space="PSUM") as psA,
            tc.tile_pool(name="psc", bufs=1, space="PSUM") as psC,
            tc.tile_pool(name="psd", bufs=1, space="PSUM") as psD,
            tc.tile_pool(name="psr", bufs=1, space="PSUM") as psR,
        ):
            # ---- persistent weights -------------------------------------
            # w_sb[n][p, kb*D + h] = W.T[kb*128 + p, h]; lhsT tile for
            # (kb, hb) = w_sb[:, kb*D + hb*128 :][:128].
            w_sb = {}
            for n in WNAMES:
                t = wp.tile([128, 2 * D], BF16, tag=n, name=n)
                nc.sync.dma_start(
                    out=t.rearrange("p (kb h) -> p kb h", kb=2),
                    in_=w_d[n].ap().rearrange("(kb p) h -> p kb h", p=128),
                )
                w_sb[n] = t

            def wsl(n, kb, hb):
                e = kb * D + hb * 128
                return w_sb[n][:, e : e + 128]

            va_sb = wp.tile([128, 2], F32, tag="va")
            nc.sync.dma_start(
                out=va_sb[:], in_=va_d.ap().rearrange("(t p) -> p t", p=128)
            )
            ones_c = wp.tile([128, 1], BF16, tag="ones_c")
            nc.vector.memset(ones_c[:], 1.0)
            ones_r = wp.tile([1, 128], BF16, tag="ones_r")
            nc.vector.memset(ones_r[:], 1.0)
            if use_bias:
                bz_sb = wp.tile([128, 2], F32, tag="bz")
                nc.sync.dma_start(out=bz_sb[:], in_=bz_d.ap())
                bh_sb = wp.tile([128, 2], F32, tag="bh")
                nc.sync.dma_start(out=bh_sb[:], in_=bh_d.ap())

            # cross-iteration tile refs for the software pipeline
            xts, hts, tzs, Es, drs, rbs, atts, htls, dds = (
                {}, {}, {}, {}, {}, {}, {}, {}, {},
            )

            def mm_pair(ps, wx, wh, rx, rh):
                """8 matmuls: ps[:, hb*ST:+ST] += Wx@rx + Wh@rh (2 k-blocks)."""
                for hb in range(2):
                    o = ps[:, hb * ST : (hb + 1) * ST]
                    for kb in range(2):
                        nc.tensor.matmul(
                            o,
                            wsl(wx, kb, hb),
                            rx[:, kb * ST : (kb + 1) * ST],
                            start=(kb == 0),
                            stop=False,
                        )
                    for kb in range(2):
                        nc.tensor.matmul(
                            o,
                            wsl(wh, kb, hb),
                            rh[:, kb * ST : (kb + 1) * ST],
                            start=False,
                            stop=(kb == 1),
                        )

            for it in range(N_ST + 3):
                # ==== loads for tile `it` ===============================
                if it < N_ST:
                    b0 = it * ST
                    xt = io.tile([128, 2 * ST], BF16, tag="xt", bufs=4)
                    nc.sync.dma_start(
                        out=xt.rearrange("p (kb b) -> p kb b", kb=2),
                        in_=x_d.ap()[:, b0 : b0 + ST].rearrange(
                            "(kb p) b -> p kb b", p=128
                        ),
                    )
                    ht = io.tile([128, 2 * ST], BF16, tag="ht", bufs=5)
                    nc.sync.dma_start(
                        out=ht.rearrange("p (kb b) -> p kb b", kb=2),
                        in_=h_d.ap()[:, b0 : b0 + ST].rearrange(
                            "(kb p) b -> p kb b", p=128
                        ),
                    )
                    xts[it], hts[it] = xt, ht

                # ==== stage C (it-2): candidate branch + tanh + sub =====
                jt = it - 2
                if 0 <= jt < N_ST:
                    xtj = xts.pop(jt)
                    attj = atts.pop(jt)
                    pc = psC.tile([128, 2 * ST], F32, tag="pc")
                    mm_pair(pc, "wht", "uht", xtj, attj)
                    htl = wk.tile([128, 2 * ST], BF16, tag="htl", bufs=3)
                    if use_bias:
                        for hb in range(2):
                            sl = slice(hb * ST, (hb + 1) * ST)
                            nc.scalar.activation(
                                htl[:, sl], pc[:, sl], AF.Tanh,
                                bias=bh_sb[:, hb : hb + 1],
                            )
                    else:
                        nc.scalar.activation(htl[:], pc[:], AF.Tanh)
                    htls[jt] = htl
                    dd = wk.tile([128, 2 * ST], BF16, tag="dd", bufs=3)
                    nc.gpsimd.tensor_sub(dd[:], htl[:], hts[jt][:])
                    dds[jt] = dd

                # ==== stage D (it-3): blend + store =====================
                bt = it - 3
                if bt >= 0:
                    htb = hts.pop(bt)
                    tzb, htlb, ddb = tzs.pop(bt), htls.pop(bt), dds.pop(bt)
                    uu = wk.tile([128, 2 * ST], BF16, tag="uu")
                    nc.vector.tensor_scalar(
                        uu[:], tzb[:], 0.5, 0.5, op0=ALU.mult, op1=ALU.add
                    )
                    mm_ = wk.tile([128, 2 * ST], BF16, tag="mm_")
                    nc.vector.tensor_mul(mm_[:], uu[:], ddb[:])
                    ot = io.tile([128, 2 * ST], BF16, tag="ot", bufs=3)
                    nc.vector.tensor_add(ot[:], mm_[:], htb[:])
                    nc.sync.dma_start(
                        out=out_d.ap()[:, bt * ST : (bt + 1) * ST].rearrange(
                            "(hb p) b -> p hb b", p=128
                        ),
                        in_=ot.rearrange("p (hb b) -> p hb b", hb=2),
                    )

                # ==== stage B (it-1): denom bcast + normalize ===========
                kt = it - 1
                if 0 <= kt < N_ST:
                    rbp = psR.tile([128, ST], F32, tag="rbp")
                    nc.tensor.matmul(
                        rbp[:], ones_r[:], drs.pop(kt)[:], start=True, stop=True
                    )
                    rb = wk.tile([128, ST], F32, tag="rb")
                    nc.vector.reciprocal_approx_fast(out=rb[:], in_=rbp[:])
                    rbs[kt] = rb
                    hr = wk.tile([128, 2 * ST], BF16, tag="hr")
                    for kb in range(2):
                        sl = slice(kb * ST, (kb + 1) * ST)
                        nc.vector.tensor_mul(hr[:, sl], hts[kt][:, sl], rb[:])
                    att = wk.tile([128, 2 * ST], BF16, tag="att", bufs=3)
                    nc.vector.tensor_mul(att[:], Es.pop(kt)[:], hr[:])
                    atts[kt] = att

                if it >= N_ST:
                    continue

                # ==== stage A: attention + z branches for tile `it` =====
                pa = psA.tile([128, 2 * ST], F32, tag="pa")
                mm_pair(pa, "wat", "uat", xt, ht)
                A = wk.tile([128, 2 * ST], BF16, tag="A")
                nc.scalar.activation(A[:], pa[:], AF.Tanh)
                E = wk.tile([128, 2 * ST], BF16, tag="E", bufs=3)
                for hb in range(2):
                    sl = slice(hb * ST, (hb + 1) * ST)
                    nc.scalar.activation(
                        E[:, sl], A[:, sl], AF.Exp, scale=va_sb[:, hb : hb + 1]
                    )
                Es[it] = E

                pz = psZ.tile([128, 2 * ST], F32, tag="pz")
                mm_pair(pz, "wzt", "uzt", xt, ht)
                tz = wk.tile([128, 2 * ST], BF16, tag="tz", bufs=4)
                if use_bias:
                    for hb in range(2):
                        sl = slice(hb * ST, (hb + 1) * ST)
                        nc.scalar.activation(
                            tz[:, sl], pz[:, sl], AF.Tanh,
                            bias=bz_sb[:, hb : hb + 1], scale=0.5,
                        )
                else:
                    nc.scalar.activation(tz[:], pz[:], AF.Tanh, scale=0.5)
                tzs[it] = tz

                # softmax denominator row [1, ST] (sum over all 256 h)
                pd = psD.tile([1, ST], F32, tag="pd")
                for hb in range(2):
                    nc.tensor.matmul(
                        pd[:],
                        ones_c[:],
                        E[:, hb * ST : (hb + 1) * ST],
                        start=(hb == 0),
                        stop=(hb == 1),
                    )
                dr = wk.tile([1, ST], BF16, tag="dr", bufs=2)
                nc.scalar.activation(dr[:], pd[:], AF.Copy)
                drs[it] = dr

    nc.compile()
    return nc


LAST_RESULTS = None


def kernel(x, h_prev, W_z, U_z, b_z, W_a, U_a, v_a, W_h, U_h, b_h):
    global LAST_RESULTS
    use_bias = bool(np.any(np.asarray(b_z)) or np.any(np.asarray(b_h)))
    key = ("nc", use_bias)
    if key not in _CACHE:
        _CACHE[key] = build_nc(use_bias)
    nc = _CACHE[key]

    bf = ml_dtypes.bfloat16
    xbf = np.asarray(x, dtype=np.float32).astype(bf)
    hbf = np.asarray(h_prev, dtype=np.float32).astype(bf)
    wmats = {
        "wzt": W_z,
        "uzt": U_z,
        "wat": W_a,
        "uat": U_a,
        "wht": W_h,
        "uht": U_h,
    }
    common = {
        n: np.ascontiguousarray(np.asarray(m, dtype=np.float32).T.astype(bf))
        for n, m in wmats.items()
    }
    common["va"] = np.ascontiguousarray(np.asarray(v_a, dtype=np.float32))
    # biases laid out [128 partition, 2 h-block]; z bias pre-scaled by 0.5
    common["bzh"] = np.ascontiguousarray(
        (0.5 * np.asarray(b_z, dtype=np.float32)).reshape(2, 128).T
    )
    common["bh"] = np.ascontiguousarray(
        np.asarray(b_h, dtype=np.float32).reshape(2, 128).T
    )

    in_maps = []
    for c in range(N_CORES):
        m = dict(common)
        m["xT"] = np.ascontiguousarray(xbf[c * B_CORE : (c + 1) * B_CORE].T)
        m["hT"] = np.ascontiguousarray(hbf[c * B_CORE : (c + 1) * B_CORE].T)
        in_maps.append(m)

    LAST_RESULTS = run_bass_kernel_spmd(nc, in_maps, core_ids=list(range(N_CORES)))
    outs = LAST_RESULTS.results
    res = np.empty((B_TOTAL, D), np.float32)
    for c in range(N_CORES):
        res[c * B_CORE : (c + 1) * B_CORE] = outs[c]["out"].T
    return res
